# revision 1
# baseline (speedup 1.0000x reference)
"""DiscreteKDE kernel for 8 Trainium2 NeuronCores.

Full computation:
    Q = 64; H_I = inv(H_bandwidth)
    Z  = (idx[:,None]-idx[None,:]) @ H_I
    KW = (1/sqrt(2pi)) * exp(-0.5 * Z*Z)
    col_sums = concat([X_probs.sum(0), Y_probs.sum(0)])     # (64,)  <- 256MB read
    T  = dot(KW.sum(0), col_sums)
    out = T * ones((256,256,256))                            # 67MB write

Per-core structure (data-parallel over n; the stream is DMA-bound ~350GB/s):
  - 15 full (128, 64x64) 2MB tiles + the exact 2120-row remainder
  - full tiles 0..12 and the low half of tile 14 -> DVE elementwise
    accumulation into a HALF-tile-wide acc (two 2048-wide adds per tile:
    same element count, but the final (p, g*q)->(p, q) fold only spans 32
    groups), folded while the last tiles are in flight
  - zero-padded tail + tile 13 + high half of tile 14 -> PE ones-matmul
    into one PSUM bank (N=512 slices accumulate on top of each other ->
    (g mod 8, q) partials)
  - Newton-Schulz inverse of H on PE+ACT (iteration on the negated inverse
    R' = R A R + 2R -- no sign bookkeeping), hidden under the stream
  - each core computes its local scalar dot d = <KW.sum(0), part>; a 4-byte
    AllGather (cheaper floor than AllReduce) + one reduce gives T = sum d_r,
    broadcast T, fill 1/8 of the output via two 4MB broadcast-AP DMAs
  - DMA rings: stream on sync; tiles 1/3/5 + tail + small consts on the
    scalar(ACT) ring only where issue can never be gated on Newton copies
"""

import os
import sys

import numpy as np

for _p in ("/opt/trn_rl_repo", "/root/.axon_site/_ro/trn_rl_repo"):
    if os.path.isdir(_p) and _p not in sys.path:
        sys.path.insert(0, _p)

import concourse.bacc as bacc
import concourse.bass as bass
import concourse.mybir as mybir
from concourse.bass_utils import run_bass_kernel_spmd
from concourse.tile import TileContext
from concourse.tile_rust import add_dep_helper

# ---- problem constants (hardcoded per spec) ----
N_TOTAL = 1_000_000
FDIM = 61
HDIM = 3
Q = 64                      # FDIM + HDIM
KGRID = 256
HOUT = 3
NCORES = 8
ROWS_PER_CORE = N_TOTAL // NCORES          # 125000

# ---- tiling ----
P = 128                     # partitions
G = 64                      # rows per partition per full tile
TILE_ROWS = P * G           # 8192
NFULL = ROWS_PER_CORE // TILE_ROWS         # 15 full tiles
TAIL_ROWS = ROWS_PER_CORE - NFULL * TILE_ROWS   # 2120
TAIL_P = 106                # 2120 = 106 partitions * 20 rows
TAIL_G = TAIL_ROWS // TAIL_P               # 20
TAIL_W = TAIL_G * Q                        # 1280
TW = G * Q                  # 4096 f32 per partition = 16KB; tile = 2MB
NDVE_FULL = 13              # full tiles 0..12 accumulate on DVE
HALF_W = TW // 2            # 2048 (half of tile 14 for each engine)

OUT_TOTAL = KGRID ** HOUT                  # 16_777_216
OUT_PER_CORE = OUT_TOTAL // NCORES         # 2_097_152
FILL_W = 2048
N_FILL = OUT_PER_CORE // (P * FILL_W)      # 8

NEWTON_ITERS = 11
INV_SQRT_2PI = 0.3989422804014327
LN_C = float(np.log(INV_SQRT_2PI))

F32 = mybir.dt.float32
AX = mybir.AxisListType
ALU = mybir.AluOpType
ACT_FN = mybir.ActivationFunctionType


def build_nc():
    nc = bacc.Bacc("TRN2", target_bir_lowering=False, debug=False,
                   num_devices=NCORES)

    c_in = nc.dram_tensor("c", [ROWS_PER_CORE, Q], F32, kind="ExternalInput")
    h_in = nc.dram_tensor("h", [Q, Q], F32, kind="ExternalInput")
    out = nc.dram_tensor("o", [OUT_PER_CORE], F32, kind="ExternalOutput")

    idx = np.arange(Q, dtype=np.float64)
    d_const = nc.inline_tensor(
        (idx[:, None] - idx[None, :]).astype(np.float32), "dmat")
    i2_const = nc.inline_tensor(
        (2.0 * np.eye(Q)).astype(np.float32), "i2mat")
    n2_const = nc.inline_tensor(
        (-2.0 * np.eye(Q)).astype(np.float32), "n2mat")

    cc_in = nc.dram_tensor("cc_in", [1], F32)
    cc_out = nc.dram_tensor("cc_out", [NCORES], F32, addr_space="Shared")

    with TileContext(nc) as tc:
        with (
            tc.tile_pool(name="const", bufs=1) as cpool,
            tc.tile_pool(name="stream", bufs=9) as spool,
            tc.tile_pool(name="small", bufs=2) as mpool,
            tc.tile_pool(name="accp", bufs=1, space=bass.MemorySpace.PSUM) as ppool,
            tc.tile_pool(name="psmall", bufs=2, space=bass.MemorySpace.PSUM) as pspool,
        ):
            # ---------- Phase A streams (emitted first; the tail goes out
            # first so its reduction is done long before the end) ----------
            cv = c_in.ap()[:NFULL * TILE_ROWS, :].rearrange(
                "(t p g) q -> t p (g q)", p=P, g=G)
            tail_v = c_in.ap()[NFULL * TILE_ROWS:, :].rearrange(
                "(p g) q -> p (g q)", p=TAIL_P, g=TAIL_G)
            TAIL_WPAD = 1536            # 3 x 512: uniform PE matmul slices
            tail_t = cpool.tile([TAIL_P, TAIL_WPAD], F32)
            nc.gpsimd.memset(tail_t[:TAIL_P, TAIL_W:], 0.0)
            nc.sync.dma_start(tail_t[:TAIL_P, :TAIL_W], tail_v)
            # all stream DMAs ride the sync ring: the scalar (ACT) engine
            # also executes the Newton copies, and in-order DMA issue there
            # would gate half the stream on the Newton chain
            # two DMA queues: sync (HWDGE) + gpsimd (SWDGE). The scalar/ACT
            # ring is left for Newton copies + small consts, so no stream DMA
            # is ever gated on compute.
            tiles = []
            for t in range(NFULL):
                st = spool.tile([P, TW], F32, tag="stream")
                if t >= NFULL - 3:
                    # split the last tiles into half-transfers: consumers
                    # start on the low half while the high half lands
                    nc.sync.dma_start(st[:, :HALF_W], cv[t][:, :HALF_W])
                    nc.sync.dma_start(st[:, HALF_W:], cv[t][:, HALF_W:])
                else:
                    nc.sync.dma_start(st[:], cv[t])
                tiles.append(st)

            # ---------- constants ----------
            ones_k = cpool.tile([P, 1], F32)        # lhsT for partition-reduce
            nc.vector.memset(ones_k[:], 1.0)
            ones_row = cpool.tile([1, P], F32)      # lhsT for bcast scalar->128
            nc.vector.memset(ones_row[:], 1.0)
            ones_q = cpool.tile([Q, 1], F32)        # lhsT for 64-partition reduce
            nc.vector.memset(ones_q[:], 1.0)
            ones_rq = cpool.tile([1, Q], F32)       # lhsT for bcast scalar->64
            nc.vector.memset(ones_rq[:], 1.0)
            lnc = cpool.tile([Q, 1], F32)           # exp bias = ln(1/sqrt(2pi))
            nc.vector.memset(lnc[:], LN_C)
            ones_fill = cpool.tile([P, FILL_W], F32)
            nc.gpsimd.memset(ones_fill[:], 1.0)

            # ---------- small inputs (SWDGE; keep HWDGE rings for the stream)
            a_t = cpool.tile([Q, Q], F32)
            nc.scalar.dma_start(a_t[:], h_in.ap())
            d_t = cpool.tile([Q, Q], F32)
            nc.scalar.dma_start(d_t[:], d_const.ap())
            i2_t = cpool.tile([Q, Q], F32)
            nc.scalar.dma_start(i2_t[:], i2_const.ap())
            n2_t = cpool.tile([Q, Q], F32)
            nc.scalar.dma_start(n2_t[:], n2_const.ap())

            # ---------- Phase B: Newton-Schulz inverse, EARLY on PE + ACT ---
            # alpha = 1 / (2*trace(A));  R0 = -I/tr = n2 * alpha
            tmp_qq = mpool.tile([Q, Q], F32, tag="qq")
            nc.gpsimd.tensor_mul(tmp_qq[:], a_t[:], i2_t[:])
            diag2 = mpool.tile([Q, 1], F32, tag="q1")
            diag2_i = nc.vector.tensor_reduce(diag2[:], tmp_qq[:], axis=AX.X,
                                              op=ALU.add)
            ps_tr = pspool.tile([1, 1], F32, tag="ps_small")
            nc.tensor.matmul(ps_tr[:], ones_q[:], diag2[:])       # 2*trace
            tr2 = mpool.tile([1, 1], F32, tag="s11")
            recip_i = nc.vector.reciprocal(tr2[:], ps_tr[:])      # 1/(2tr)
            ps_a = pspool.tile([Q, 1], F32, tag="ps_small")
            nc.tensor.matmul(ps_a[:], ones_rq[:], tr2[:])         # bcast->(64,1)
            al64 = mpool.tile([Q, 1], F32, tag="q1b")
            nc.scalar.activation(al64[:], ps_a[:], ACT_FN.Copy)
            # Iterate on R_k = -X_k:  R' = R A R + 2R  (no sign alternation;
            # R -> -H^-1 and only Z^2 is used downstream).
            s_cur = mpool.tile([Q, Q], F32, tag="newton")
            nc.vector.tensor_scalar_mul(s_cur[:], n2_t[:], al64[:])
            for _ in range(NEWTON_ITERS):
                ps_y = pspool.tile([Q, Q], F32, tag="ps_qq")
                nc.tensor.matmul(ps_y[:], a_t[:], s_cur[:])       # A @ R
                y_sb = mpool.tile([Q, Q], F32, tag="newton_y")
                nc.scalar.activation(y_sb[:], ps_y[:], ACT_FN.Copy)
                ps_x = pspool.tile([Q, Q], F32, tag="ps_qq")
                nc.tensor.matmul(ps_x[:], s_cur[:], y_sb[:],
                                 start=True, stop=False)          # R A R
                nc.tensor.matmul(ps_x[:], s_cur[:], i2_t[:],
                                 start=False, stop=True)          # + 2 R
                s_nxt = mpool.tile([Q, Q], F32, tag="newton")
                nc.scalar.activation(s_nxt[:], ps_x[:], ACT_FN.Copy)
                s_cur = s_nxt
            # Z = D.T @ (-H^-1) up to sign; KW = exp(-Z^2/2 + ln c)
            ps_z = pspool.tile([Q, Q], F32, tag="ps_qq")
            nc.tensor.matmul(ps_z[:], d_t[:], s_cur[:])
            z2 = mpool.tile([Q, Q], F32, tag="qq")
            nc.scalar.square(z2[:], ps_z[:])
            kw = mpool.tile([Q, Q], F32, tag="qq2")
            nc.scalar.activation(kw[:], z2[:], ACT_FN.Exp,
                                 bias=lnc[:], scale=-0.5)
            ps_s = pspool.tile([1, Q], F32, tag="ps_small")
            nc.tensor.matmul(ps_s[:], ones_q[:], kw[:])           # KW.sum(0)
            s_sb = mpool.tile([1, Q], F32, tag="vec2")
            nc.scalar.activation(s_sb[:], ps_s[:], ACT_FN.Copy)

            # ---------- Phase A compute ----------
            # DVE: a pure streaming chain - accumulate full tiles 0..12,
            # fold, reduce half of tile 14; nothing else sits on DVE before
            # the adds, so the chain tracks the DMA arrivals exactly.
            # acc is HALF a tile wide: each tile contributes its two
            # 2048-wide halves (same DVE element count), so the final
            # (p, g*q) -> (p, q) fold only spans 32 groups (3.6us not 7us),
            # and tile 14's low half just joins the adds.
            acc = cpool.tile([P, HALF_W], F32)
            add_insts = []
            for t in range(NDVE_FULL):
                if t == 0:
                    ai = nc.vector.tensor_copy(acc[:], tiles[t][:, :HALF_W])
                else:
                    ai = nc.vector.tensor_add(acc[:], acc[:],
                                              tiles[t][:, :HALF_W])
                add_insts.append(ai)
                nc.vector.tensor_add(acc[:], acc[:], tiles[t][:, HALF_W:])
            # keep the tiny alpha-chain DVE ops out of the front of the DVE
            # queue: their upstream (gpsimd/PE) deps would otherwise stall
            # the in-order engine before the first adds, gating DMA slot
            # reuse and with it the whole stream
            add_dep_helper(diag2_i.ins, add_insts[1].ins,
                           reason="alpha chain yields to stream adds")
            add_dep_helper(recip_i.ins, add_insts[2].ins,
                           reason="alpha chain yields to stream adds")
            nc.vector.tensor_add(acc[:], acc[:], tiles[14][:, :HALF_W])
            acc2 = cpool.tile([P, Q], F32)
            acc_v = acc[:].rearrange("p (g q) -> p q g", g=G // 2, q=Q)
            nc.vector.tensor_reduce(acc2[:], acc_v, axis=AX.X, op=ALU.add)
            # PE: tail tile (early) + tile 13 + second half of tile 14 via
            # mod-8 ones-matmuls into one PSUM bank
            ps_pe = ppool.tile([1, 512], F32)
            pe_slices = [(tail_t[:TAIL_P, b * 512:(b + 1) * 512], TAIL_P)
                         for b in range(3)]
            pe_slices += [(tiles[13][:, b * 512:(b + 1) * 512], P)
                          for b in range(TW // 512)]
            pe_slices += [(tiles[14][:, HALF_W + b * 512:
                                     HALF_W + (b + 1) * 512], P)
                          for b in range(HALF_W // 512)]
            for i, (sl, np_) in enumerate(pe_slices):
                nc.tensor.matmul(ps_pe[:], ones_k[:np_, :], sl,
                                 start=(i == 0), stop=(i == len(pe_slices) - 1))
            # partition-reduce the DVE-side partials on PE
            ps2 = ppool.tile([1, Q], F32)
            nc.tensor.matmul(ps2[:], ones_k[:], acc2[:])
            # fold PE psum (1, 8*q) -> (1, q) on DVE and merge both halves
            pe_fold = mpool.tile([1, Q], F32, tag="vec0")
            ps_pe_v = ps_pe[:].rearrange("p (g q) -> p q g", g=8, q=Q)
            nc.vector.tensor_reduce(pe_fold[:], ps_pe_v, axis=AX.X, op=ALU.add)
            part = mpool.tile([1, Q], F32, tag="vec")
            nc.vector.tensor_add(part[:], pe_fold[:], ps2[:])

            # ---------- Phase C: AllGather the per-core scalar dot ----
            # T = sum_r dot(s, part_r); the dot distributes over ranks, so
            # ship 4 bytes instead of the 64-vector and the post-collective
            # chain collapses to one tiny reduce.
            dprod = mpool.tile([1, Q], F32, tag="vec2b")
            nc.vector.tensor_mul(dprod[:], s_sb[:], part[:])
            d_loc = mpool.tile([1, 1], F32, tag="s11d")
            nc.vector.tensor_reduce(d_loc[:], dprod[:], axis=AX.X, op=ALU.add)
            nc.sync.dma_start(cc_in.ap(), d_loc[:])
            nc.gpsimd.collective_compute(
                "AllGather", ALU.bypass,
                replica_groups=[list(range(NCORES))],
                ins=[cc_in.ap()], outs=[cc_out.ap()],
            )
            gath = mpool.tile([1, NCORES], F32, tag="gath")
            nc.sync.dma_start(gath[:], cc_out.ap())

            # ---------- Phase D: T = sum_r d_r; fill ----------
            t_sc = mpool.tile([1, 1], F32, tag="s11c")
            nc.vector.tensor_reduce(t_sc[:], gath[:], axis=AX.X, op=ALU.add)
            ps_b = pspool.tile([P, 1], F32, tag="ps_small")
            nc.tensor.matmul(ps_b[:], ones_row[:], t_sc[:])       # bcast->(128,1)
            tb = mpool.tile([P, 1], F32, tag="q1c")
            nc.scalar.activation(tb[:], ps_b[:], ACT_FN.Copy)
            fill = cpool.tile([P, FILL_W], F32)
            nc.vector.tensor_scalar_mul(fill[:], ones_fill[:], tb[:])
            # fill: one broadcast DMA per HWDGE ring (sync + scalar), each 4MB
            half = N_FILL // 2
            ovh = out.ap().rearrange("(h j p f) -> h p j f", h=2, p=P, f=FILL_W)
            fill_b = fill[:].unsqueeze(1).broadcast_to([P, half, FILL_W])
            nc.sync.dma_start(ovh[0], fill_b)
            nc.scalar.dma_start(ovh[1], fill_b)

    nc.compile()
    return nc


_NC_CACHE = None


def _get_nc():
    global _NC_CACHE
    if _NC_CACHE is None:
        _NC_CACHE = build_nc()
    return _NC_CACHE


def run(X_probs, Y_probs, H_bandwidth, trace=False, trace_kwargs=None):
    X = np.asarray(X_probs, dtype=np.float32).reshape(NCORES, ROWS_PER_CORE, FDIM)
    Y = np.asarray(Y_probs, dtype=np.float32).reshape(NCORES, ROWS_PER_CORE, HDIM)
    H = np.ascontiguousarray(np.asarray(H_bandwidth, dtype=np.float32))

    C = np.empty((NCORES, ROWS_PER_CORE, Q), dtype=np.float32)
    C[:, :, :FDIM] = X
    C[:, :, FDIM:] = Y

    nc = _get_nc()
    in_maps = [{"c": C[i], "h": H} for i in range(NCORES)]
    res = run_bass_kernel_spmd(nc, in_maps, list(range(NCORES)),
                               trace=trace, **(trace_kwargs or {}))
    full = np.concatenate([res.results[i]["o"] for i in range(NCORES)])
    return full.reshape((KGRID,) * HOUT), res


def kernel(X_probs, Y_probs, H_bandwidth, K, H_out):
    assert int(K) == KGRID and int(H_out) == HOUT
    out, _ = run(X_probs, Y_probs, H_bandwidth, trace=False)
    return out

